# revision 11
# baseline (speedup 1.0000x reference)
"""AlignmentContrastiveLoss on 8 TRN2 NeuronCores (Bass/Tile, SPMD).

scores[b,c] = sum_j max_i (im[b,1+i,:] . s[c,1+j,:]) over valid i<im_len[b]-1,
j<s_len[c]-3 (the max also includes 0 whenever b has any invalid i), followed
by a diagonal-margin contrastive loss over the [B,B] score matrix.

Strategy:
  - Host: slice, permute the batch (loss is invariant under a joint b/c
    permutation), snake-deal images to 8 cores sorted by length, pack valid
    image regions into per-core slot columns (bf16; every slot holding a
    short image keeps >=1 zero pad column so the reduce reproduces the
    reference max-with-0), pack valid sentence words globally (bf16,
    replicated), word->sentence indicator blocks (bf16).
  - Device: stationary = 128-word S blocks, moving = packed im columns;
    bf16 matmuls accumulate over D in PSUM; DVE segmented max over slot
    width classes; bf16 indicator matmul (stationary = the [128,16] maxv)
    accumulates scores[b_local, c]; scores are AllGathered in two chunks
    (the first hidden under compute, after its sentence columns finalize);
    every core then computes the full margin loss redundantly and writes
    the same scalar.
"""

import numpy as np

import concourse.bass as bass
import concourse.bacc as bacc
import concourse.tile as tile
import concourse.mybir as mybir
from concourse import bass_utils

try:
    from ml_dtypes import bfloat16, float8_e4m3
except ImportError:  # jax ships ml_dtypes
    from jax.numpy import bfloat16, float8_e4m3

N_CORES = 8
MARGIN = 0.2
DEBUG = False  # adds a "dbg" output with the gathered score matrix


def _choose_classes(widths):
    """Partition sorted-desc slot widths into classes (round width up to the
    class value). DP minimizing: per-run DVE overhead ~125ns + per-pad-col
    ~3.0ns (fp8 PE stream + DVE element)."""
    ws = sorted(widths, reverse=True)
    n = len(ws)
    RUN = 125.0
    PAD = 3.0
    INF = float("inf")
    dp = [INF] * (n + 1)
    dp[n] = 0.0
    choice = [0] * n
    for i in range(n - 1, -1, -1):
        w = ws[i]
        for j in range(i + 1, n + 1):
            pad = sum(w - ws[k] for k in range(i, j))
            c = RUN + PAD * pad + dp[j]
            if c < dp[i]:
                dp[i] = c
                choice[i] = j
    out = []
    i = 0
    while i < n:
        j = choice[i]
        out.append((i, j - i, ws[i]))  # (slot_start, count, width)
        i = j
    return out


def _prepare(im_set, s_seq, im_len, s_len):
    """Host-side shard/pack. Returns (meta, in_maps)."""
    im_set = np.ascontiguousarray(np.asarray(im_set, dtype=np.float32))
    s_seq = np.ascontiguousarray(np.asarray(s_seq, dtype=np.float32))
    im_l = np.asarray(im_len).astype(np.int64) - 1
    s_l = np.asarray(s_len).astype(np.int64) - 3

    B = im_set.shape[0]
    D = im_set.shape[2]
    Li = im_set.shape[1] - 1
    Ls = s_seq.shape[1] - 3
    R = B // N_CORES

    im = im_set[:, 1:, :]
    s = s_seq[:, 1 : 1 + Ls, :]
    im_l = np.clip(im_l, 0, Li)
    s_l = np.clip(s_l, 0, Ls)

    # --- permute batch: sort by im_l desc, snake-deal to cores ---
    order = np.argsort(-im_l, kind="stable")
    assign = [[] for _ in range(N_CORES)]
    for idx, b in enumerate(order):
        rnd, pos = divmod(idx, N_CORES)
        core = pos if rnd % 2 == 0 else N_CORES - 1 - pos
        assign[core].append(int(b))
    sigma = np.array([b for m in range(N_CORES) for b in assign[m]])

    # --- slot widths (shared across cores) ---
    # effective width forces >=1 zero pad for short images so the reduce's
    # max includes 0 exactly as the reference's zero-masked tail does
    imls = np.array(
        [[im_l[assign[m][r]] for r in range(R)] for m in range(N_CORES)]
    )  # [cores, R]
    eff = np.minimum(imls + (imls < Li), Li)
    wmax = eff.max(axis=0)  # [R], non-increasing
    runs = _choose_classes(list(wmax))
    slot_w = np.zeros(R, np.int64)
    for r0, nr, wdt in runs:
        slot_w[r0 : r0 + nr] = wdt
    assert np.all(slot_w >= wmax)
    offs = np.concatenate([[0], np.cumsum(slot_w)]).astype(np.int64)
    n_im = int(offs[-1])
    n_im = (n_im + 15) // 16 * 16  # DoubleRow moving AP wants 16-aligned

    # segments of slots with cumulative width <= 512 (PSUM bank limit;
    # keep headroom for the 16-alignment pad on the last segment)
    segs = []  # (slot_lo, slot_hi, col_lo, col_hi)
    lo = 0
    for r in range(R + 1):
        if r == R or offs[r + 1] - offs[lo] > 496:
            hi_col = n_im if r == R else int(offs[r])
            segs.append((lo, r, int(offs[lo]), hi_col))
            lo = r
    assert segs[-1][1] == R and segs[-1][3] == n_im

    # --- per-core moving operand [128 (D part), 8 (D chunk), n_im] fp8 ---
    imt_cores = []
    for m in range(N_CORES):
        imt = np.zeros((D, n_im), np.float32)
        for r in range(R):
            b = assign[m][r]
            L = int(im_l[b])
            imt[:, offs[r] : offs[r] + L] = im[b, :L, :].T
        imt = imt.astype(float8_e4m3).reshape(8, 128, n_im).transpose(1, 0, 2)
        imt_cores.append(np.ascontiguousarray(imt))

    # --- packed sentence words, sigma order ---
    n_words = int(s_l.sum())
    G = (n_words + 127) // 128
    w_pad = G * 128
    s_pack = np.zeros((w_pad, D), np.float32)
    word_c = np.full(w_pad, -1, np.int64)
    w = 0
    cum = np.zeros(B + 1, np.int64)  # words before sentence c (sigma order)
    for p in range(B):
        c_old = sigma[p]
        L = int(s_l[c_old])
        cum[p] = w
        s_pack[w : w + L] = s[c_old, :L, :]
        word_c[w : w + L] = p
        w += L
    cum[B] = w

    # per-block stream: [G, 128 (part), 8*128 (s chunks)] fp8
    sb = s_pack.astype(float8_e4m3).reshape(G, 128, 8, 128)  # [g, w, k, kp]
    blk = np.ascontiguousarray(sb.transpose(0, 3, 2, 1).reshape(G, 128, -1))
    # indicator, resident: [128 (word-in-block), G, B (c)] bf16
    ind = np.zeros((G, 128, B), bfloat16)
    gs, ws_ = np.divmod(np.arange(w_pad), 128)
    valid = word_c >= 0
    ind[gs[valid], ws_[valid], word_c[valid]] = 1.0
    ind = np.ascontiguousarray(ind.transpose(1, 0, 2))  # [128, G, B]

    # early-gather splits: columns [0, c_k) have all their words inside the
    # first g_k blocks, so their score accumulators finalize early and can
    # be AllGathered under the remaining compute; the last chunk is small
    chunks = []  # (col_lo, col_hi, g_first, g_last)
    bounds = sorted({max(1, (2 * G) // 3), G})
    prev_c = 0
    for gk in bounds:
        ck = int(np.sum(cum[1:] <= gk * 128)) if gk < G else B
        ck = max(0, min(ck, B))
        if ck - prev_c < 8 and gk < G:
            continue
        if gk == G:
            ck = B
        if ck <= prev_c:
            continue
        gfirst = int(cum[prev_c] // 128)
        chunks.append((prev_c, ck, gfirst, gk))
        prev_c = ck
    if not chunks or chunks[-1][1] != B:
        chunks = [(0, B, 0, G)]

    eye = np.ascontiguousarray(np.eye(B, dtype=np.float32))

    meta = dict(B=B, D=D, R=R, n_im=n_im, G=G, runs=runs, segs=segs,
                offs=offs, chunks=chunks)
    blk = blk.reshape(G, 128, 8, 128)
    in_maps = []
    for m in range(N_CORES):
        in_maps.append(
            {"imt": imt_cores[m], "blk": blk, "ind": ind, "eye": eye}
        )
    return meta, in_maps


def _build(meta):
    B, R, n_im, G = meta["B"], meta["R"], meta["n_im"], meta["G"]
    runs, segs, offs = meta["runs"], meta["segs"], meta["offs"]
    chunks = meta["chunks"]
    f32, bf16 = mybir.dt.float32, mybir.dt.bfloat16
    fp8 = mybir.dt.float8e4
    KC = meta["D"] // 128  # contraction chunks
    GRP = [list(range(N_CORES))]

    nc = bacc.Bacc("TRN2", target_bir_lowering=False, debug=False,
                   num_devices=N_CORES)
    imt_d = nc.dram_tensor("imt", [128, KC, n_im], fp8, kind="ExternalInput")
    blk_d = nc.dram_tensor("blk", [G, 128, KC, 128], fp8,
                           kind="ExternalInput")
    ind_d = nc.dram_tensor("ind", [128, G, B], bf16, kind="ExternalInput")
    eye_d = nc.dram_tensor("eye", [B, B], f32, kind="ExternalInput")
    out_d = nc.dram_tensor("out", [1, 1], f32, kind="ExternalOutput")
    dbg_d = (
        nc.dram_tensor("dbg", [128, B], bf16, kind="ExternalOutput")
        if DEBUG
        else None
    )

    # runs per segment (split any run that crosses a segment boundary)
    seg_runs = []
    for (slo, shi, clo, chi) in segs:
        rr = []
        for r0, nr, wdt in runs:
            lo, hi = max(r0, slo), min(r0 + nr, shi)
            if lo < hi:
                rr.append((lo, hi - lo, wdt))
        seg_runs.append(rr)

    # score columns are accumulated in one PSUM tile per gather chunk so
    # each chunk's last write lands at its final block (tile-granular deps)

    with tile.TileContext(nc) as tc:
        with (
            tc.tile_pool(name="resident", bufs=1) as resident,
            tc.tile_pool(name="blkp", bufs=5) as blk_pool,
            tc.tile_pool(name="maxv", bufs=G + 2) as maxv_pool,
            tc.tile_pool(
                name="ps",
                bufs=max(2, min(4, (6 - len(chunks)) // len(segs))),
                space="PSUM",
            ) as ps_pool,
            tc.tile_pool(name="psS", bufs=1, space="PSUM") as psS_pool,
            tc.tile_pool(name="tail", bufs=1) as tailp,
            tc.tile_pool(name="dram", bufs=1, space="DRAM") as dram,
        ):
            maxv_list = []
            pe_order_dep = None  # order ind batch before later blocks on PE
            pending_cp = None
            last_gp = [None]  # last gpsimd instr, to chain gpsimd order

            def gp_chain(inst):
                if last_gp[0] is not None:
                    tile.add_dep_helper(inst.ins, last_gp[0].ins, sync=False,
                                        reason="gpsimd order")
                last_gp[0] = inst

            # dummy collective first: absorbs the CC bootstrap barrier /
            # peer rendezvous during the preamble so the real gathers
            # later start immediately on their doorbells
            dmy_sb = tailp.tile([1, 16], bf16, name="dmy")
            nc.gpsimd.memset(dmy_sb[:], 0.0)
            dmy_in = dram.tile([1, 16], bf16, name="dmyin", tag="dmyin")
            dm0 = nc.gpsimd.dma_start(out=dmy_in[:], in_=dmy_sb[:])
            gp_chain(dm0)
            dmy_out = dram.tile([N_CORES, 1, 16], bf16, name="dmyout",
                                tag="dmyout")
            cc0 = nc.gpsimd.collective_compute(
                "AllGather", mybir.AluOpType.bypass, replica_groups=GRP,
                ins=[dmy_in[:].opt()], outs=[dmy_out[:].opt()],
            )
            gp_chain(cc0)

            # resident tiles
            imt_sb = resident.tile([128, KC, n_im], fp8)
            for k in range(KC):
                nc.scalar.dma_start(out=imt_sb[:, k, :], in_=imt_d[:, k, :])
            ind_sb = resident.tile([128, G, B], bf16)
            nc.scalar.dma_start(out=ind_sb[:], in_=ind_d[:])
            eye_sb = resident.tile([B, B], f32)
            nc.scalar.dma_start(out=eye_sb[:], in_=eye_d[:])
            ones_sb = resident.tile([B, B], f32)
            nc.gpsimd.memset(ones_sb[:], 1.0)
            ones_col = resident.tile([128, 1], f32)
            nc.gpsimd.memset(ones_col[:], 1.0)

            # one score accumulator + gather buffers per column chunk
            score_tiles = []
            for ci, (clo, chi, gfirst, glast) in enumerate(chunks):
                score_tiles.append(
                    psS_pool.tile(
                        [R, chi - clo], f32, name=f"sc{ci}", tag=f"sc{ci}"
                    )
                )
            gather_bufs = []
            t_sb = tailp.tile([128, B], bf16)  # T[b, c] (bf16 transport)

            def emit_gather(ci):
                clo, chi, _, _ = chunks[ci]
                w = chi - clo
                part_sb = tailp.tile([R, w], bf16, name=f"part{ci}", tag=f"part{ci}")
                cp = nc.vector.tensor_copy(part_sb[:], score_tiles[ci][:])
                cin = dram.tile([R, w], bf16, name=f"cin{ci}", tag=f"cin{ci}")
                dm = nc.gpsimd.dma_start(out=cin[:], in_=part_sb[:])
                gp_chain(dm)
                cout = dram.tile([N_CORES, R, w], bf16, name=f"cout{ci}", tag=f"cout{ci}")
                cc = nc.gpsimd.collective_compute(
                    "AllGather", mybir.AluOpType.bypass, replica_groups=GRP,
                    ins=[cin[:].opt()], outs=[cout[:].opt()],
                )
                gp_chain(cc)
                # assemble this chunk's T[b, c] columns as soon as the
                # gather lands (Scalar engine is idle; chunk 0's assembly
                # then happens mid-loop, off the critical tail)
                nc.scalar.dma_start(
                    out=t_sb[:, clo:chi],
                    in_=cout[:].rearrange("m b c -> (m b) c"),
                )
                gather_bufs.append((clo, chi, cout))
                return cp

            def emit_ind_batch(ci, maxv_list):
                # all of chunk ci's indicator matmuls back-to-back: the PE
                # main stream stays pure MM431 (keeps HAM un-throttled)
                clo, chi, gfirst, glast = chunks[ci]
                last = None
                for g in range(gfirst, glast):
                    last = nc.tensor.matmul(
                        score_tiles[ci][:],
                        maxv_list[g][:],
                        ind_sb[:, g, clo:chi],
                        start=(g == gfirst),
                        stop=(g == glast - 1),
                    )
                return last

            for g in range(G):
                blk_sb = blk_pool.tile([128, KC, 128], fp8)
                nc.sync.dma_start(out=blk_sb[:], in_=blk_d[g])

                ps_tiles = []
                for si, (slo, shi, clo, chi) in enumerate(segs):
                    ps = ps_pool.tile([128, chi - clo], f32, tag=f"ps{si}")
                    ps_tiles.append(ps)
                first_mm = None
                for k in range(0, KC, 2):
                    for si, (slo, shi, clo, chi) in enumerate(segs):
                        mm = nc.tensor.matmul(
                            ps_tiles[si][:],
                            blk_sb[:, k : k + 2, :],
                            imt_sb[:, k : k + 2, clo:chi],
                            start=(k == 0),
                            stop=(k == KC - 2),
                            perf_mode=mybir.MatmulPerfMode.DoubleRow,
                        )
                        if first_mm is None:
                            first_mm = mm
                if pe_order_dep is not None:
                    tile.add_dep_helper(first_mm.ins, pe_order_dep.ins,
                                        sync=False, reason="ind batch order")
                    pe_order_dep = None

                maxv = maxv_pool.tile([128, R], bf16)
                first_red = None
                for si, (slo, shi, clo, chi) in enumerate(segs):
                    for r0, nr, wdt in seg_runs[si]:
                        base = int(offs[r0]) - clo
                        src = ps_tiles[si][:, base : base + nr * wdt]
                        red = nc.vector.tensor_reduce(
                            maxv[:, r0 : r0 + nr],
                            src.rearrange("p (n w) -> p n w", w=wdt),
                            axis=mybir.AxisListType.X,
                            op=mybir.AluOpType.max,
                        )
                        if first_red is None:
                            first_red = red
                maxv_list.append(maxv)
                if pending_cp is not None:
                    # keep the gather copy ahead of later blocks' reduces
                    # in the DVE stream
                    tile.add_dep_helper(first_red.ins, pending_cp.ins,
                                        sync=False, reason="gather cp order")
                    pending_cp = None

                for ci in range(len(chunks) - 1):
                    if g == chunks[ci][3] - 1:
                        # chunk ci is final: run its ind batch + gather
                        # now, hidden under the remaining blocks' compute
                        pe_order_dep = emit_ind_batch(ci, maxv_list)
                        pending_cp = emit_gather(ci)

            emit_ind_batch(len(chunks) - 1, maxv_list)
            emit_gather(len(chunks) - 1)

            # ---- loss tail on every core (t_sb assembled per gather) ----
            if DEBUG:
                nc.sync.dma_start(out=dbg_d[:], in_=t_sb[:])

            masked = tailp.tile([128, B], f32)
            nc.vector.tensor_tensor(
                masked[:], t_sb[:], eye_sb[:], op=mybir.AluOpType.mult
            )
            diag_col = tailp.tile([128, 1], f32)
            nc.vector.tensor_reduce(
                diag_col[:], masked[:], axis=mybir.AxisListType.X,
                op=mybir.AluOpType.add,
            )
            # mneg = MARGIN - diag[b]
            mneg = tailp.tile([128, 1], f32)
            nc.vector.tensor_scalar(
                mneg[:], diag_col[:], -1.0, MARGIN,
                op0=mybir.AluOpType.mult, op1=mybir.AluOpType.add,
            )
            # cost_s = relu(T + (margin - diag[b]))  (per-partition scalar)
            sum_s = tailp.tile([128, 1], f32)
            tmp_s = tailp.tile([128, B], f32)
            nc.vector.tensor_scalar(
                tmp_s[:], t_sb[:], mneg[:, 0:1], 0.0,
                op0=mybir.AluOpType.add, op1=mybir.AluOpType.max,
            )
            nc.vector.tensor_reduce(
                sum_s[:], tmp_s[:], axis=mybir.AxisListType.X,
                op=mybir.AluOpType.add,
            )
            # gmat[b, c] = diag[c] via ones^T @ masked
            gmat_ps = psS_pool.tile([128, B], f32)
            nc.tensor.matmul(gmat_ps[:], ones_sb[:], masked[:], start=True,
                             stop=True)
            tmp_i = tailp.tile([128, B], f32)
            nc.vector.tensor_tensor(
                tmp_i[:], t_sb[:], gmat_ps[:], op=mybir.AluOpType.subtract
            )
            sum_i = tailp.tile([128, 1], f32)
            tmp_i2 = tailp.tile([128, B], f32)
            nc.vector.tensor_scalar(
                tmp_i2[:], tmp_i[:], MARGIN, 0.0,
                op0=mybir.AluOpType.add, op1=mybir.AluOpType.max,
            )
            nc.vector.tensor_reduce(
                sum_i[:], tmp_i2[:], axis=mybir.AxisListType.X,
                op=mybir.AluOpType.add,
            )
            tot = tailp.tile([128, 1], f32)
            nc.vector.tensor_tensor(
                tot[:], sum_s[:], sum_i[:], op=mybir.AluOpType.add
            )
            # partition sum via matmul: [1,1] = tot^T @ ones_col
            fin_ps = psS_pool.tile([1, 1], f32)
            nc.tensor.matmul(fin_ps[:], tot[:], ones_col[:], start=True,
                             stop=True)
            res_sb = tailp.tile([1, 1], f32)
            # subtract the diagonal contribution 2*B*MARGIN
            nc.vector.tensor_scalar(
                res_sb[:], fin_ps[:], -2.0 * B * MARGIN, None,
                op0=mybir.AluOpType.add,
            )
            nc.sync.dma_start(out=out_d[:], in_=res_sb[:])

    nc.compile()
    return nc


def run(im_set, s_seq, im_len, s_len, trace=False):
    meta, in_maps = _prepare(im_set, s_seq, im_len, s_len)
    nc = _build(meta)
    res = bass_utils.run_bass_kernel_spmd(
        nc, in_maps, core_ids=list(range(N_CORES)), trace=trace
    )
    val = np.float32(res.results[0]["out"][0, 0])
    return np.asarray(val, dtype=np.float32).reshape(()), res


def kernel(im_set, s_seq, im_len, s_len):
    out, _ = run(im_set, s_seq, im_len, s_len, trace=False)
    return out



# revision 13
# speedup vs baseline: 1.2156x; 1.2156x over previous
"""AlignmentContrastiveLoss on 8 TRN2 NeuronCores (Bass/Tile, SPMD).

scores[b,c] = sum_j max_i (im[b,1+i,:] . s[c,1+j,:]) over valid i<im_len[b]-1,
j<s_len[c]-3 (the max also includes 0 whenever b has any invalid i), followed
by a diagonal-margin contrastive loss over the [B,B] score matrix.

Strategy:
  - Host: slice, permute the batch (loss is invariant under a joint b/c
    permutation), snake-deal images to 8 cores sorted by length, pack valid
    image regions into per-core slot columns (bf16; every slot holding a
    short image keeps >=1 zero pad column so the reduce reproduces the
    reference max-with-0), pack valid sentence words globally (bf16,
    replicated), word->sentence indicator blocks (bf16).
  - Device: stationary = 128-word S blocks, moving = packed im columns;
    bf16 matmuls accumulate over D in PSUM; DVE segmented max over slot
    width classes; bf16 indicator matmul (stationary = the [128,16] maxv)
    accumulates scores[b_local, c]; scores are AllGathered in two chunks
    (the first hidden under compute, after its sentence columns finalize);
    every core then computes the full margin loss redundantly and writes
    the same scalar.
"""

import numpy as np

import concourse.bass as bass
import concourse.bacc as bacc
import concourse.tile as tile
import concourse.mybir as mybir
from concourse import bass_utils

try:
    from ml_dtypes import bfloat16, float8_e4m3
except ImportError:  # jax ships ml_dtypes
    from jax.numpy import bfloat16, float8_e4m3

N_CORES = 8
MARGIN = 0.2
DEBUG = False  # adds a "dbg" output with the gathered score matrix


def _choose_classes(widths):
    """Partition sorted-desc slot widths into classes (round width up to the
    class value). DP minimizing: per-run DVE overhead ~125ns + per-pad-col
    ~3.0ns (fp8 PE stream + DVE element)."""
    ws = sorted(widths, reverse=True)
    n = len(ws)
    RUN = 125.0
    PAD = 3.0
    INF = float("inf")
    dp = [INF] * (n + 1)
    dp[n] = 0.0
    choice = [0] * n
    for i in range(n - 1, -1, -1):
        w = ws[i]
        for j in range(i + 1, n + 1):
            pad = sum(w - ws[k] for k in range(i, j))
            c = RUN + PAD * pad + dp[j]
            if c < dp[i]:
                dp[i] = c
                choice[i] = j
    out = []
    i = 0
    while i < n:
        j = choice[i]
        out.append((i, j - i, ws[i]))  # (slot_start, count, width)
        i = j
    return out


def _prepare(im_set, s_seq, im_len, s_len):
    """Host-side shard/pack. Returns (meta, in_maps)."""
    im_set = np.ascontiguousarray(np.asarray(im_set, dtype=np.float32))
    s_seq = np.ascontiguousarray(np.asarray(s_seq, dtype=np.float32))
    im_l = np.asarray(im_len).astype(np.int64) - 1
    s_l = np.asarray(s_len).astype(np.int64) - 3

    B = im_set.shape[0]
    D = im_set.shape[2]
    Li = im_set.shape[1] - 1
    Ls = s_seq.shape[1] - 3
    R = B // N_CORES

    im = im_set[:, 1:, :]
    s = s_seq[:, 1 : 1 + Ls, :]
    im_l = np.clip(im_l, 0, Li)
    s_l = np.clip(s_l, 0, Ls)

    # --- permute batch: sort by im_l desc, snake-deal to cores ---
    order = np.argsort(-im_l, kind="stable")
    assign = [[] for _ in range(N_CORES)]
    for idx, b in enumerate(order):
        rnd, pos = divmod(idx, N_CORES)
        core = pos if rnd % 2 == 0 else N_CORES - 1 - pos
        assign[core].append(int(b))
    sigma = np.array([b for m in range(N_CORES) for b in assign[m]])

    # --- slot widths (shared across cores) ---
    # effective width forces >=1 zero pad for short images so the reduce's
    # max includes 0 exactly as the reference's zero-masked tail does
    imls = np.array(
        [[im_l[assign[m][r]] for r in range(R)] for m in range(N_CORES)]
    )  # [cores, R]
    eff = np.minimum(imls + (imls < Li), Li)
    wmax = eff.max(axis=0)  # [R], non-increasing
    runs = _choose_classes(list(wmax))
    slot_w = np.zeros(R, np.int64)
    for r0, nr, wdt in runs:
        slot_w[r0 : r0 + nr] = wdt
    assert np.all(slot_w >= wmax)
    offs = np.concatenate([[0], np.cumsum(slot_w)]).astype(np.int64)
    n_im = int(offs[-1])
    n_im = (n_im + 15) // 16 * 16  # DoubleRow moving AP wants 16-aligned

    # segments of slots with cumulative width <= 512 (PSUM bank limit;
    # keep headroom for the 16-alignment pad on the last segment)
    segs = []  # (slot_lo, slot_hi, col_lo, col_hi)
    lo = 0
    for r in range(R + 1):
        if r == R or offs[r + 1] - offs[lo] > 496:
            hi_col = n_im if r == R else int(offs[r])
            segs.append((lo, r, int(offs[lo]), hi_col))
            lo = r
    assert segs[-1][1] == R and segs[-1][3] == n_im

    # --- per-core moving operand [128 (D part), 8 (D chunk), n_im] fp8 ---
    imt_cores = []
    for m in range(N_CORES):
        imt = np.zeros((D, n_im), np.float32)
        for r in range(R):
            b = assign[m][r]
            L = int(im_l[b])
            imt[:, offs[r] : offs[r] + L] = im[b, :L, :].T
        imt = imt.astype(float8_e4m3).reshape(8, 128, n_im).transpose(1, 0, 2)
        imt_cores.append(np.ascontiguousarray(imt))

    # --- packed sentence words, sigma order ---
    n_words = int(s_l.sum())
    G = (n_words + 127) // 128
    w_pad = G * 128
    s_pack = np.zeros((w_pad, D), np.float32)
    word_c = np.full(w_pad, -1, np.int64)
    w = 0
    cum = np.zeros(B + 1, np.int64)  # words before sentence c (sigma order)
    for p in range(B):
        c_old = sigma[p]
        L = int(s_l[c_old])
        cum[p] = w
        s_pack[w : w + L] = s[c_old, :L, :]
        word_c[w : w + L] = p
        w += L
    cum[B] = w

    # per-block stream: [G, 128 (part), 8*128 (s chunks)] fp8
    sb = s_pack.astype(float8_e4m3).reshape(G, 128, 8, 128)  # [g, w, k, kp]
    blk = np.ascontiguousarray(sb.transpose(0, 3, 2, 1).reshape(G, 128, -1))
    # indicator, resident: [128 (word-in-block), G, B (c)] bf16
    ind = np.zeros((G, 128, B), bfloat16)
    gs, ws_ = np.divmod(np.arange(w_pad), 128)
    valid = word_c >= 0
    ind[gs[valid], ws_[valid], word_c[valid]] = 1.0
    ind = np.ascontiguousarray(ind.transpose(1, 0, 2))  # [128, G, B]

    # early-gather splits: columns [0, c_k) have all their words inside the
    # first g_k blocks, so their score accumulators finalize early and can
    # be AllGathered under the remaining compute; the last chunk is small
    chunks = []  # (col_lo, col_hi, g_first, g_last)
    bounds = sorted({G})
    prev_c = 0
    for gk in bounds:
        ck = int(np.sum(cum[1:] <= gk * 128)) if gk < G else B
        ck = max(0, min(ck, B))
        if ck - prev_c < 8 and gk < G:
            continue
        if gk == G:
            ck = B
        if ck <= prev_c:
            continue
        gfirst = int(cum[prev_c] // 128)
        chunks.append((prev_c, ck, gfirst, gk))
        prev_c = ck
    if not chunks or chunks[-1][1] != B:
        chunks = [(0, B, 0, G)]

    eye = np.ascontiguousarray(np.eye(B, dtype=np.float32))

    meta = dict(B=B, D=D, R=R, n_im=n_im, G=G, runs=runs, segs=segs,
                offs=offs, chunks=chunks)
    blk = blk.reshape(G, 128, 8, 128)
    in_maps = []
    for m in range(N_CORES):
        in_maps.append(
            {"imt": imt_cores[m], "blk": blk, "ind": ind, "eye": eye}
        )
    return meta, in_maps


def _build(meta):
    B, R, n_im, G = meta["B"], meta["R"], meta["n_im"], meta["G"]
    runs, segs, offs = meta["runs"], meta["segs"], meta["offs"]
    chunks = meta["chunks"]
    f32, bf16 = mybir.dt.float32, mybir.dt.bfloat16
    fp8 = mybir.dt.float8e4
    KC = meta["D"] // 128  # contraction chunks
    GRP = [list(range(N_CORES))]

    nc = bacc.Bacc("TRN2", target_bir_lowering=False, debug=False,
                   num_devices=N_CORES)
    imt_d = nc.dram_tensor("imt", [128, KC, n_im], fp8, kind="ExternalInput")
    blk_d = nc.dram_tensor("blk", [G, 128, KC, 128], fp8,
                           kind="ExternalInput")
    ind_d = nc.dram_tensor("ind", [128, G, B], bf16, kind="ExternalInput")
    eye_d = nc.dram_tensor("eye", [B, B], f32, kind="ExternalInput")
    out_d = nc.dram_tensor("out", [1, 1], f32, kind="ExternalOutput")
    dbg_d = (
        nc.dram_tensor("dbg", [128, B], bf16, kind="ExternalOutput")
        if DEBUG
        else None
    )

    # runs per segment (split any run that crosses a segment boundary)
    seg_runs = []
    for (slo, shi, clo, chi) in segs:
        rr = []
        for r0, nr, wdt in runs:
            lo, hi = max(r0, slo), min(r0 + nr, shi)
            if lo < hi:
                rr.append((lo, hi - lo, wdt))
        seg_runs.append(rr)

    # score columns are accumulated in one PSUM tile per gather chunk so
    # each chunk's last write lands at its final block (tile-granular deps)

    with tile.TileContext(nc) as tc:
        with (
            tc.tile_pool(name="resident", bufs=1) as resident,
            tc.tile_pool(name="blkp", bufs=5) as blk_pool,
            tc.tile_pool(name="maxv", bufs=G + 2) as maxv_pool,
            tc.tile_pool(
                name="ps",
                bufs=max(2, min(4, (6 - len(chunks)) // len(segs))),
                space="PSUM",
            ) as ps_pool,
            tc.tile_pool(name="psS", bufs=1, space="PSUM") as psS_pool,
            tc.tile_pool(name="tail", bufs=1) as tailp,
            tc.tile_pool(name="dram", bufs=1, space="DRAM") as dram,
        ):
            maxv_list = []
            pe_order_dep = None  # order ind batch before later blocks on PE
            pending_cp = None
            last_gp = [None]  # last gpsimd instr, to chain gpsimd order

            def gp_chain(inst):
                if last_gp[0] is not None:
                    tile.add_dep_helper(inst.ins, last_gp[0].ins, sync=False,
                                        reason="gpsimd order")
                last_gp[0] = inst

            # resident tiles
            imt_sb = resident.tile([128, KC, n_im], fp8)
            for k in range(KC):
                nc.scalar.dma_start(out=imt_sb[:, k, :], in_=imt_d[:, k, :])
            ind_sb = resident.tile([128, G, B], bf16)
            nc.scalar.dma_start(out=ind_sb[:], in_=ind_d[:])
            eye_sb = resident.tile([B, B], f32)
            nc.scalar.dma_start(out=eye_sb[:], in_=eye_d[:])
            ones_sb = resident.tile([B, B], f32)
            nc.gpsimd.memset(ones_sb[:], 1.0)
            ones_col = resident.tile([128, 1], f32)
            nc.gpsimd.memset(ones_col[:], 1.0)

            # one score accumulator + gather buffers per column chunk
            score_tiles = []
            for ci, (clo, chi, gfirst, glast) in enumerate(chunks):
                score_tiles.append(
                    psS_pool.tile(
                        [R, chi - clo], f32, name=f"sc{ci}", tag=f"sc{ci}"
                    )
                )
            gather_bufs = []
            t_sb = tailp.tile([128, B], bf16)  # T[b, c] (bf16 transport)

            def emit_gather(ci):
                clo, chi, _, _ = chunks[ci]
                w = chi - clo
                part_sb = tailp.tile([R, w], bf16, name=f"part{ci}", tag=f"part{ci}")
                cp = nc.vector.tensor_copy(part_sb[:], score_tiles[ci][:])
                cin = dram.tile([R, w], bf16, name=f"cin{ci}", tag=f"cin{ci}")
                dm = nc.gpsimd.dma_start(out=cin[:], in_=part_sb[:])
                gp_chain(dm)
                cout = dram.tile([N_CORES, R, w], bf16, name=f"cout{ci}", tag=f"cout{ci}")
                cc = nc.gpsimd.collective_compute(
                    "AllGather", mybir.AluOpType.bypass, replica_groups=GRP,
                    ins=[cin[:].opt()], outs=[cout[:].opt()],
                )
                gp_chain(cc)
                # assemble this chunk's T[b, c] columns as soon as the
                # gather lands (Scalar engine is idle; chunk 0's assembly
                # then happens mid-loop, off the critical tail)
                nc.scalar.dma_start(
                    out=t_sb[:, clo:chi],
                    in_=cout[:].rearrange("m b c -> (m b) c"),
                )
                gather_bufs.append((clo, chi, cout))
                return cp

            def emit_ind_batch(ci, maxv_list):
                # all of chunk ci's indicator matmuls back-to-back: the PE
                # main stream stays pure MM431 (keeps HAM un-throttled)
                clo, chi, gfirst, glast = chunks[ci]
                last = None
                for g in range(gfirst, glast):
                    last = nc.tensor.matmul(
                        score_tiles[ci][:],
                        maxv_list[g][:],
                        ind_sb[:, g, clo:chi],
                        start=(g == gfirst),
                        stop=(g == glast - 1),
                    )
                return last

            for g in range(G):
                blk_sb = blk_pool.tile([128, KC, 128], fp8)
                nc.sync.dma_start(out=blk_sb[:], in_=blk_d[g])

                ps_tiles = []
                for si, (slo, shi, clo, chi) in enumerate(segs):
                    ps = ps_pool.tile([128, chi - clo], f32, tag=f"ps{si}")
                    ps_tiles.append(ps)
                first_mm = None
                for k in range(0, KC, 2):
                    for si, (slo, shi, clo, chi) in enumerate(segs):
                        mm = nc.tensor.matmul(
                            ps_tiles[si][:],
                            blk_sb[:, k : k + 2, :],
                            imt_sb[:, k : k + 2, clo:chi],
                            start=(k == 0),
                            stop=(k == KC - 2),
                            perf_mode=mybir.MatmulPerfMode.DoubleRow,
                        )
                        if first_mm is None:
                            first_mm = mm
                if pe_order_dep is not None:
                    tile.add_dep_helper(first_mm.ins, pe_order_dep.ins,
                                        sync=False, reason="ind batch order")
                    pe_order_dep = None

                maxv = maxv_pool.tile([128, R], bf16)
                first_red = None
                for si, (slo, shi, clo, chi) in enumerate(segs):
                    for r0, nr, wdt in seg_runs[si]:
                        base = int(offs[r0]) - clo
                        src = ps_tiles[si][:, base : base + nr * wdt]
                        red = nc.vector.tensor_reduce(
                            maxv[:, r0 : r0 + nr],
                            src.rearrange("p (n w) -> p n w", w=wdt),
                            axis=mybir.AxisListType.X,
                            op=mybir.AluOpType.max,
                        )
                        if first_red is None:
                            first_red = red
                maxv_list.append(maxv)
                if pending_cp is not None:
                    # keep the gather copy ahead of later blocks' reduces
                    # in the DVE stream
                    tile.add_dep_helper(first_red.ins, pending_cp.ins,
                                        sync=False, reason="gather cp order")
                    pending_cp = None

                for ci in range(len(chunks) - 1):
                    if g == chunks[ci][3] - 1:
                        # chunk ci is final: run its ind batch + gather
                        # now, hidden under the remaining blocks' compute
                        pe_order_dep = emit_ind_batch(ci, maxv_list)
                        pending_cp = emit_gather(ci)

            emit_ind_batch(len(chunks) - 1, maxv_list)
            emit_gather(len(chunks) - 1)

            # ---- loss tail on every core (t_sb assembled per gather) ----
            if DEBUG:
                nc.sync.dma_start(out=dbg_d[:], in_=t_sb[:])

            masked = tailp.tile([128, B], f32)
            nc.vector.tensor_tensor(
                masked[:], t_sb[:], eye_sb[:], op=mybir.AluOpType.mult
            )
            diag_col = tailp.tile([128, 1], f32)
            nc.vector.tensor_reduce(
                diag_col[:], masked[:], axis=mybir.AxisListType.X,
                op=mybir.AluOpType.add,
            )
            # mneg = MARGIN - diag[b]
            mneg = tailp.tile([128, 1], f32)
            nc.vector.tensor_scalar(
                mneg[:], diag_col[:], -1.0, MARGIN,
                op0=mybir.AluOpType.mult, op1=mybir.AluOpType.add,
            )
            # cost_s = relu(T + (margin - diag[b]))  (per-partition scalar)
            sum_s = tailp.tile([128, 1], f32)
            tmp_s = tailp.tile([128, B], f32)
            nc.vector.tensor_scalar(
                tmp_s[:], t_sb[:], mneg[:, 0:1], 0.0,
                op0=mybir.AluOpType.add, op1=mybir.AluOpType.max,
            )
            nc.vector.tensor_reduce(
                sum_s[:], tmp_s[:], axis=mybir.AxisListType.X,
                op=mybir.AluOpType.add,
            )
            # gmat[b, c] = diag[c] via ones^T @ masked
            gmat_ps = psS_pool.tile([128, B], f32)
            nc.tensor.matmul(gmat_ps[:], ones_sb[:], masked[:], start=True,
                             stop=True)
            tmp_i = tailp.tile([128, B], f32)
            nc.vector.tensor_tensor(
                tmp_i[:], t_sb[:], gmat_ps[:], op=mybir.AluOpType.subtract
            )
            sum_i = tailp.tile([128, 1], f32)
            tmp_i2 = tailp.tile([128, B], f32)
            nc.vector.tensor_scalar(
                tmp_i2[:], tmp_i[:], MARGIN, 0.0,
                op0=mybir.AluOpType.add, op1=mybir.AluOpType.max,
            )
            nc.vector.tensor_reduce(
                sum_i[:], tmp_i2[:], axis=mybir.AxisListType.X,
                op=mybir.AluOpType.add,
            )
            tot = tailp.tile([128, 1], f32)
            nc.vector.tensor_tensor(
                tot[:], sum_s[:], sum_i[:], op=mybir.AluOpType.add
            )
            # partition sum via matmul: [1,1] = tot^T @ ones_col
            fin_ps = psS_pool.tile([1, 1], f32)
            nc.tensor.matmul(fin_ps[:], tot[:], ones_col[:], start=True,
                             stop=True)
            res_sb = tailp.tile([1, 1], f32)
            # subtract the diagonal contribution 2*B*MARGIN
            nc.vector.tensor_scalar(
                res_sb[:], fin_ps[:], -2.0 * B * MARGIN, None,
                op0=mybir.AluOpType.add,
            )
            nc.sync.dma_start(out=out_d[:], in_=res_sb[:])

    nc.compile()
    return nc


def run(im_set, s_seq, im_len, s_len, trace=False):
    meta, in_maps = _prepare(im_set, s_seq, im_len, s_len)
    nc = _build(meta)
    res = bass_utils.run_bass_kernel_spmd(
        nc, in_maps, core_ids=list(range(N_CORES)), trace=trace
    )
    val = np.float32(res.results[0]["out"][0, 0])
    return np.asarray(val, dtype=np.float32).reshape(()), res


def kernel(im_set, s_seq, im_len, s_len):
    out, _ = run(im_set, s_seq, im_len, s_len, trace=False)
    return out



# revision 14
# speedup vs baseline: 1.3382x; 1.1009x over previous
"""AlignmentContrastiveLoss on 8 TRN2 NeuronCores (Bass/Tile, SPMD).

scores[b,c] = sum_j max_i (im[b,1+i,:] . s[c,1+j,:]) over valid i<im_len[b]-1,
j<s_len[c]-3 (the max also includes 0 whenever b has any invalid i), followed
by a diagonal-margin contrastive loss over the [B,B] score matrix.

Strategy:
  - Host: slice, permute the batch (loss is invariant under a joint b/c
    permutation), snake-deal images to 8 cores sorted by length, pack valid
    image regions into per-core slot columns (fp8e4; every slot holding a
    short image keeps >=1 zero pad column so the reduce reproduces the
    reference max-with-0), pack valid sentence words globally (fp8e4,
    replicated), word->sentence indicator blocks (bf16).
  - Device: stationary = 128-word S blocks, moving = packed im columns;
    fp8 DoubleRow matmuls (256-deep contraction per pass, 2x PE rate)
    accumulate over D in PSUM; DVE segmented max over slot width classes;
    bf16 indicator matmul (stationary = the [128,16] maxv) accumulates
    scores[b_local, c]; one AllGather of the [16,128] bf16 score rows at
    the end (collective latency is dominated by the CC rendezvous of the
    slowest core, so chunked/early gathers buy nothing); every core then
    computes the full margin loss redundantly and writes the same scalar.
"""

import numpy as np

import concourse.bass as bass
import concourse.bacc as bacc
import concourse.tile as tile
import concourse.mybir as mybir
from concourse import bass_utils

try:
    from ml_dtypes import bfloat16, float8_e4m3
except ImportError:  # jax ships ml_dtypes
    from jax.numpy import bfloat16, float8_e4m3

N_CORES = 8
MARGIN = 0.2
DEBUG = False  # adds a "dbg" output with the gathered score matrix


def _choose_classes(widths):
    """Partition sorted-desc slot widths into classes (round width up to the
    class value). DP minimizing: per-run DVE overhead ~125ns + per-pad-col
    ~3.0ns (fp8 PE stream + DVE element)."""
    ws = sorted(widths, reverse=True)
    n = len(ws)
    RUN = 125.0
    PAD = 3.0
    INF = float("inf")
    dp = [INF] * (n + 1)
    dp[n] = 0.0
    choice = [0] * n
    for i in range(n - 1, -1, -1):
        w = ws[i]
        for j in range(i + 1, n + 1):
            pad = sum(w - ws[k] for k in range(i, j))
            c = RUN + PAD * pad + dp[j]
            if c < dp[i]:
                dp[i] = c
                choice[i] = j
    out = []
    i = 0
    while i < n:
        j = choice[i]
        out.append((i, j - i, ws[i]))  # (slot_start, count, width)
        i = j
    return out


def _prepare(im_set, s_seq, im_len, s_len):
    """Host-side shard/pack. Returns (meta, in_maps)."""
    im_set = np.ascontiguousarray(np.asarray(im_set, dtype=np.float32))
    s_seq = np.ascontiguousarray(np.asarray(s_seq, dtype=np.float32))
    im_l = np.asarray(im_len).astype(np.int64) - 1
    s_l = np.asarray(s_len).astype(np.int64) - 3

    B = im_set.shape[0]
    D = im_set.shape[2]
    Li = im_set.shape[1] - 1
    Ls = s_seq.shape[1] - 3
    R = B // N_CORES

    im = im_set[:, 1:, :]
    s = s_seq[:, 1 : 1 + Ls, :]
    im_l = np.clip(im_l, 0, Li)
    s_l = np.clip(s_l, 0, Ls)

    # --- permute batch: sort by im_l desc, snake-deal to cores ---
    order = np.argsort(-im_l, kind="stable")
    assign = [[] for _ in range(N_CORES)]
    for idx, b in enumerate(order):
        rnd, pos = divmod(idx, N_CORES)
        core = pos if rnd % 2 == 0 else N_CORES - 1 - pos
        assign[core].append(int(b))
    sigma = np.array([b for m in range(N_CORES) for b in assign[m]])

    # --- slot widths (shared across cores) ---
    # effective width forces >=1 zero pad for short images so the reduce's
    # max includes 0 exactly as the reference's zero-masked tail does
    imls = np.array(
        [[im_l[assign[m][r]] for r in range(R)] for m in range(N_CORES)]
    )  # [cores, R]
    eff = np.minimum(imls + (imls < Li), Li)
    wmax = eff.max(axis=0)  # [R], non-increasing
    runs = _choose_classes(list(wmax))
    slot_w = np.zeros(R, np.int64)
    for r0, nr, wdt in runs:
        slot_w[r0 : r0 + nr] = wdt
    assert np.all(slot_w >= wmax)
    offs = np.concatenate([[0], np.cumsum(slot_w)]).astype(np.int64)
    n_im = int(offs[-1])
    n_im = (n_im + 15) // 16 * 16  # DoubleRow moving AP wants 16-aligned

    # segments of slots with cumulative width <= 512 (PSUM bank limit;
    # keep headroom for the 16-alignment pad on the last segment)
    segs = []  # (slot_lo, slot_hi, col_lo, col_hi)
    lo = 0
    for r in range(R + 1):
        if r == R or offs[r + 1] - offs[lo] > 496:
            hi_col = n_im if r == R else int(offs[r])
            segs.append((lo, r, int(offs[lo]), hi_col))
            lo = r
    assert segs[-1][1] == R and segs[-1][3] == n_im

    # --- per-core moving operand [128 (D part), 8 (D chunk), n_im] fp8 ---
    imt_cores = []
    for m in range(N_CORES):
        imt = np.zeros((D, n_im), np.float32)
        for r in range(R):
            b = assign[m][r]
            L = int(im_l[b])
            imt[:, offs[r] : offs[r] + L] = im[b, :L, :].T
        imt = imt.astype(float8_e4m3).reshape(8, 128, n_im).transpose(1, 0, 2)
        imt_cores.append(np.ascontiguousarray(imt))

    # --- packed sentence words, sigma order ---
    n_words = int(s_l.sum())
    G = (n_words + 127) // 128
    w_pad = G * 128
    s_pack = np.zeros((w_pad, D), np.float32)
    word_c = np.full(w_pad, -1, np.int64)
    w = 0
    cum = np.zeros(B + 1, np.int64)  # words before sentence c (sigma order)
    for p in range(B):
        c_old = sigma[p]
        L = int(s_l[c_old])
        cum[p] = w
        s_pack[w : w + L] = s[c_old, :L, :]
        word_c[w : w + L] = p
        w += L
    cum[B] = w

    # per-block stream: [G, 128 (part), 8*128 (s chunks)] fp8
    sb = s_pack.astype(float8_e4m3).reshape(G, 128, 8, 128)  # [g, w, k, kp]
    blk = np.ascontiguousarray(sb.transpose(0, 3, 2, 1).reshape(G, 128, -1))
    # indicator, resident: [128 (word-in-block), G, B (c)] bf16
    ind = np.zeros((G, 128, B), bfloat16)
    gs, ws_ = np.divmod(np.arange(w_pad), 128)
    valid = word_c >= 0
    ind[gs[valid], ws_[valid], word_c[valid]] = 1.0
    ind = np.ascontiguousarray(ind.transpose(1, 0, 2))  # [128, G, B]

    # early-gather splits: columns [0, c_k) have all their words inside the
    # first g_k blocks, so their score accumulators finalize early and can
    # be AllGathered under the remaining compute; the last chunk is small
    chunks = []  # (col_lo, col_hi, g_first, g_last)
    bounds = sorted({G})
    prev_c = 0
    for gk in bounds:
        ck = int(np.sum(cum[1:] <= gk * 128)) if gk < G else B
        ck = max(0, min(ck, B))
        if ck - prev_c < 8 and gk < G:
            continue
        if gk == G:
            ck = B
        if ck <= prev_c:
            continue
        gfirst = int(cum[prev_c] // 128)
        chunks.append((prev_c, ck, gfirst, gk))
        prev_c = ck
    if not chunks or chunks[-1][1] != B:
        chunks = [(0, B, 0, G)]

    eye = np.ascontiguousarray(np.eye(B, dtype=np.float32))

    meta = dict(B=B, D=D, R=R, n_im=n_im, G=G, runs=runs, segs=segs,
                offs=offs, chunks=chunks)
    blk = blk.reshape(G, 128, 8, 128)
    in_maps = []
    for m in range(N_CORES):
        in_maps.append(
            {"imt": imt_cores[m], "blk": blk, "ind": ind, "eye": eye}
        )
    return meta, in_maps


def _build(meta):
    B, R, n_im, G = meta["B"], meta["R"], meta["n_im"], meta["G"]
    runs, segs, offs = meta["runs"], meta["segs"], meta["offs"]
    chunks = meta["chunks"]
    f32, bf16 = mybir.dt.float32, mybir.dt.bfloat16
    fp8 = mybir.dt.float8e4
    KC = meta["D"] // 128  # contraction chunks
    GRP = [list(range(N_CORES))]

    nc = bacc.Bacc("TRN2", target_bir_lowering=False, debug=False,
                   num_devices=N_CORES)
    imt_d = nc.dram_tensor("imt", [128, KC, n_im], fp8, kind="ExternalInput")
    blk_d = nc.dram_tensor("blk", [G, 128, KC, 128], fp8,
                           kind="ExternalInput")
    ind_d = nc.dram_tensor("ind", [128, G, B], bf16, kind="ExternalInput")
    eye_d = nc.dram_tensor("eye", [B, B], f32, kind="ExternalInput")
    out_d = nc.dram_tensor("out", [1, 1], f32, kind="ExternalOutput")
    dbg_d = (
        nc.dram_tensor("dbg", [128, B], bf16, kind="ExternalOutput")
        if DEBUG
        else None
    )

    # runs per segment (split any run that crosses a segment boundary)
    seg_runs = []
    for (slo, shi, clo, chi) in segs:
        rr = []
        for r0, nr, wdt in runs:
            lo, hi = max(r0, slo), min(r0 + nr, shi)
            if lo < hi:
                rr.append((lo, hi - lo, wdt))
        seg_runs.append(rr)

    # score columns are accumulated in one PSUM tile per gather chunk so
    # each chunk's last write lands at its final block (tile-granular deps)

    with tile.TileContext(nc) as tc:
        with (
            tc.tile_pool(name="resident", bufs=1) as resident,
            tc.tile_pool(name="blkp", bufs=5) as blk_pool,
            tc.tile_pool(name="maxv", bufs=G + 2) as maxv_pool,
            tc.tile_pool(
                name="ps",
                bufs=max(2, min(4, (6 - len(chunks)) // len(segs))),
                space="PSUM",
            ) as ps_pool,
            tc.tile_pool(name="psS", bufs=1, space="PSUM") as psS_pool,
            tc.tile_pool(name="tail", bufs=1) as tailp,
            tc.tile_pool(name="dram", bufs=1, space="DRAM") as dram,
        ):
            maxv_list = []
            pe_order_dep = None  # order ind batch before later blocks on PE
            pending_cp = None
            last_gp = [None]  # last gpsimd instr, to chain gpsimd order

            def gp_chain(inst):
                if last_gp[0] is not None:
                    tile.add_dep_helper(inst.ins, last_gp[0].ins, sync=False,
                                        reason="gpsimd order")
                last_gp[0] = inst

            # resident tiles
            imt_sb = resident.tile([128, KC, n_im], fp8)
            for k in range(KC):
                nc.scalar.dma_start(out=imt_sb[:, k, :], in_=imt_d[:, k, :])
            ind_sb = resident.tile([128, G, B], bf16)
            nc.scalar.dma_start(out=ind_sb[:], in_=ind_d[:])
            eye_sb = resident.tile([B, B], f32)
            nc.scalar.dma_start(out=eye_sb[:], in_=eye_d[:])
            ones_sb = resident.tile([B, B], f32)
            nc.gpsimd.memset(ones_sb[:], 1.0)
            ones_col = resident.tile([128, 1], f32)
            nc.gpsimd.memset(ones_col[:], 1.0)

            # one score accumulator + gather buffers per column chunk
            score_tiles = []
            for ci, (clo, chi, gfirst, glast) in enumerate(chunks):
                score_tiles.append(
                    psS_pool.tile(
                        [R, chi - clo], f32, name=f"sc{ci}", tag=f"sc{ci}"
                    )
                )
            gather_bufs = []
            t_sb = tailp.tile([128, B], bf16)  # T[b, c] (bf16 transport)

            def emit_gather(ci):
                clo, chi, _, _ = chunks[ci]
                w = chi - clo
                part_sb = tailp.tile([R, w], bf16, name=f"part{ci}", tag=f"part{ci}")
                cp = nc.vector.tensor_copy(part_sb[:], score_tiles[ci][:])
                cin = dram.tile([R, w], bf16, name=f"cin{ci}", tag=f"cin{ci}")
                dm = nc.gpsimd.dma_start(out=cin[:], in_=part_sb[:])
                gp_chain(dm)
                cout = dram.tile([N_CORES, R, w], bf16, name=f"cout{ci}", tag=f"cout{ci}")
                cc = nc.gpsimd.collective_compute(
                    "AllGather", mybir.AluOpType.bypass, replica_groups=GRP,
                    ins=[cin[:].opt()], outs=[cout[:].opt()],
                )
                gp_chain(cc)
                # assemble this chunk's T[b, c] columns as soon as the
                # gather lands (Scalar engine is idle; chunk 0's assembly
                # then happens mid-loop, off the critical tail)
                nc.scalar.dma_start(
                    out=t_sb[:, clo:chi],
                    in_=cout[:].rearrange("m b c -> (m b) c"),
                )
                gather_bufs.append((clo, chi, cout))
                return cp

            def emit_ind_batch(ci, maxv_list):
                # all of chunk ci's indicator matmuls back-to-back: the PE
                # main stream stays pure MM431 (keeps HAM un-throttled)
                clo, chi, gfirst, glast = chunks[ci]
                last = None
                for g in range(gfirst, glast):
                    last = nc.tensor.matmul(
                        score_tiles[ci][:],
                        maxv_list[g][:],
                        ind_sb[:, g, clo:chi],
                        start=(g == gfirst),
                        stop=(g == glast - 1),
                    )
                return last

            for g in range(G):
                blk_sb = blk_pool.tile([128, KC, 128], fp8)
                nc.sync.dma_start(out=blk_sb[:], in_=blk_d[g])

                ps_tiles = []
                for si, (slo, shi, clo, chi) in enumerate(segs):
                    ps = ps_pool.tile([128, chi - clo], f32, tag=f"ps{si}")
                    ps_tiles.append(ps)
                first_mm = None
                for k in range(0, KC, 2):
                    for si, (slo, shi, clo, chi) in enumerate(segs):
                        mm = nc.tensor.matmul(
                            ps_tiles[si][:],
                            blk_sb[:, k : k + 2, :],
                            imt_sb[:, k : k + 2, clo:chi],
                            start=(k == 0),
                            stop=(k == KC - 2),
                            perf_mode=mybir.MatmulPerfMode.DoubleRow,
                        )
                        if first_mm is None:
                            first_mm = mm
                if pe_order_dep is not None:
                    tile.add_dep_helper(first_mm.ins, pe_order_dep.ins,
                                        sync=False, reason="ind batch order")
                    pe_order_dep = None

                maxv = maxv_pool.tile([128, R], bf16)
                first_red = None
                for si, (slo, shi, clo, chi) in enumerate(segs):
                    for r0, nr, wdt in seg_runs[si]:
                        base = int(offs[r0]) - clo
                        src = ps_tiles[si][:, base : base + nr * wdt]
                        red = nc.vector.tensor_reduce(
                            maxv[:, r0 : r0 + nr],
                            src.rearrange("p (n w) -> p n w", w=wdt),
                            axis=mybir.AxisListType.X,
                            op=mybir.AluOpType.max,
                        )
                        if first_red is None:
                            first_red = red
                maxv_list.append(maxv)
                if pending_cp is not None:
                    # keep the gather copy ahead of later blocks' reduces
                    # in the DVE stream
                    tile.add_dep_helper(first_red.ins, pending_cp.ins,
                                        sync=False, reason="gather cp order")
                    pending_cp = None

                for ci in range(len(chunks) - 1):
                    if g == chunks[ci][3] - 1:
                        # chunk ci is final: run its ind batch + gather
                        # now, hidden under the remaining blocks' compute
                        pe_order_dep = emit_ind_batch(ci, maxv_list)
                        pending_cp = emit_gather(ci)

            emit_ind_batch(len(chunks) - 1, maxv_list)
            emit_gather(len(chunks) - 1)

            # ---- loss tail on every core (t_sb assembled per gather) ----
            if DEBUG:
                nc.sync.dma_start(out=dbg_d[:], in_=t_sb[:])

            masked = tailp.tile([128, B], f32)
            nc.vector.tensor_tensor(
                masked[:], t_sb[:], eye_sb[:], op=mybir.AluOpType.mult
            )
            diag_col = tailp.tile([128, 1], f32)
            nc.vector.tensor_reduce(
                diag_col[:], masked[:], axis=mybir.AxisListType.X,
                op=mybir.AluOpType.add,
            )
            # mneg = MARGIN - diag[b]
            mneg = tailp.tile([128, 1], f32)
            nc.vector.tensor_scalar(
                mneg[:], diag_col[:], -1.0, MARGIN,
                op0=mybir.AluOpType.mult, op1=mybir.AluOpType.add,
            )
            # cost_s = relu(T + (margin - diag[b]))  (per-partition scalar)
            sum_s = tailp.tile([128, 1], f32)
            tmp_s = tailp.tile([128, B], f32)
            nc.vector.tensor_scalar(
                tmp_s[:], t_sb[:], mneg[:, 0:1], 0.0,
                op0=mybir.AluOpType.add, op1=mybir.AluOpType.max,
            )
            nc.vector.tensor_reduce(
                sum_s[:], tmp_s[:], axis=mybir.AxisListType.X,
                op=mybir.AluOpType.add,
            )
            # gmat[b, c] = diag[c] via ones^T @ masked
            gmat_ps = psS_pool.tile([128, B], f32)
            nc.tensor.matmul(gmat_ps[:], ones_sb[:], masked[:], start=True,
                             stop=True)
            tmp_i = tailp.tile([128, B], f32)
            nc.vector.tensor_tensor(
                tmp_i[:], t_sb[:], gmat_ps[:], op=mybir.AluOpType.subtract
            )
            sum_i = tailp.tile([128, 1], f32)
            tmp_i2 = tailp.tile([128, B], f32)
            nc.vector.tensor_scalar(
                tmp_i2[:], tmp_i[:], MARGIN, 0.0,
                op0=mybir.AluOpType.add, op1=mybir.AluOpType.max,
            )
            nc.vector.tensor_reduce(
                sum_i[:], tmp_i2[:], axis=mybir.AxisListType.X,
                op=mybir.AluOpType.add,
            )
            tot = tailp.tile([128, 1], f32)
            nc.vector.tensor_tensor(
                tot[:], sum_s[:], sum_i[:], op=mybir.AluOpType.add
            )
            # partition sum via matmul: [1,1] = tot^T @ ones_col
            fin_ps = psS_pool.tile([1, 1], f32)
            nc.tensor.matmul(fin_ps[:], tot[:], ones_col[:], start=True,
                             stop=True)
            res_sb = tailp.tile([1, 1], f32)
            # subtract the diagonal contribution 2*B*MARGIN
            nc.vector.tensor_scalar(
                res_sb[:], fin_ps[:], -2.0 * B * MARGIN, None,
                op0=mybir.AluOpType.add,
            )
            nc.sync.dma_start(out=out_d[:], in_=res_sb[:])

    nc.compile()
    return nc


def run(im_set, s_seq, im_len, s_len, trace=False):
    meta, in_maps = _prepare(im_set, s_seq, im_len, s_len)
    nc = _build(meta)
    res = bass_utils.run_bass_kernel_spmd(
        nc, in_maps, core_ids=list(range(N_CORES)), trace=trace
    )
    val = np.float32(res.results[0]["out"][0, 0])
    return np.asarray(val, dtype=np.float32).reshape(()), res


def kernel(im_set, s_seq, im_len, s_len):
    out, _ = run(im_set, s_seq, im_len, s_len, trace=False)
    return out

